# revision 25
# baseline (speedup 1.0000x reference)
"""Trainium2 Bass kernel v7 for nn_AttentionBlock (4x256x64x64 self-attention).

Sharding: 8 cores = 4 batches x 2 query-halves. Per core (batch b, half h):
  k    = fold_bn(Wk) @ x[b] + bk'       [64, 4096] keys, bf16
  n_j  = ||k_j||^2                      row max of E is always the diagonal
                                        here, so -n (split bf16 hi+lo) rides
                                        as 2 extra contraction channels
  E'   = [k;1;1]^T [k;-nhi;-nlo]        E'_ij = <k_i,k_j> - n_j  (K=66)
  P    = exp(E') in fp8 e4m3            E' <= ~0 so P in (0, ~1]; exp on Act
                                        (native, fp8 out) or DVE (Schraudolph
                                        u8 bit trick; rint+saturate verified)
  vT8  = fp8(x[b]^T @ Wv^T)             [4096, 258] fp8, cols 256/257 = ones
  num  = P^T @ vT8                      DoubleRow fp8 matmuls: 2 key chunks
                                        per instruction at 2x bf16 rate
  num += dv                             dv = vT - fp8(vT) on the query rows
                                        repairs fp8(v) on the diagonal
                                        (P_ii ~ 1); off-diag mass is ~3%
Host divides num[:, :256] by num[:, 256] (denominator), adds bv, reshapes.
Bit-level numpy sim of this pipeline: rel err 4.1e-3 (gate 2e-2).

Notes from measurement: DoubleRow streams 1 output col/cycle (2 fp8 input
cols/cycle), so it only pays when the pair dim carries two real contraction
chunks (PV); fp8 E gains nothing. --enable-ldw-opt=true crashes walrus.
GpSimd/Pool cannot touch PSUM. Compute-engine APs need 32-aligned partition
starts (the -n rows reach partitions 64-65 via a small DMA).
"""

import numpy as np

import concourse.bass as bass
import concourse.bacc as bacc
import concourse.tile as tile
import concourse.mybir as mybir
from concourse.bass_utils import run_bass_kernel_spmd

B, C, HH, WW = 4, 256, 64, 64
HW = HH * WW          # 4096
CK, CV = 64, 256
CKE = CK + 2          # 66: k channels + [-n_hi; -n_lo] (rhs) / [1; 1] (lhsT)
P = 128
QH = HW // 2          # 2048 queries per core
NCORES = 8
BN_EPS = 1e-5

NJ = HW // P          # 32 key chunks
NJP = NJ // 2         # 16 key chunk pairs (DoubleRow granule)
IBS = 512             # query columns per E tile
NIB = QH // IBS       # 4 i-blocks
NQ = IBS // P         # 4 query chunks of 128 per i-block
KC = 512              # hw chunk for the k/v projection matmuls
QKC = QH // KC        # 4 query projection chunks

LAG_P = 2             # PV lags behind exp by this many pairs
PRO_P = 12            # E pairs emitted during the projection prologue

F32 = mybir.dt.float32
BF16 = mybir.dt.bfloat16
FP8 = mybir.dt.float8e4
U8 = mybir.dt.uint8
EXP = mybir.ActivationFunctionType.Exp
IDENT = mybir.ActivationFunctionType.Identity
MUL = mybir.AluOpType.mult
ADD = mybir.AluOpType.add
SUB = mybir.AluOpType.subtract
DR = mybir.MatmulPerfMode.DoubleRow

# Schraudolph exp to e4m3 bits: u8 = rint(E*8/ln2 + (7*8 - 0.344)),
# f32->u8 cast on DVE verified on hw to round-to-nearest and saturate.
SCH_A = 8.0 / float(np.log(2.0))
SCH_B = 56.0 - 0.344


# exp engine per pair index: 'a' = Act (native Exp), 'v' = DVE (bit trick).
def exp_engine_for(pp):
    if pp < PRO_P:
        return 'v' if pp % 4 == 2 else 'a'
    return 'v' if pp % 2 == 1 else 'a'


def _emit(tc, xb, wkT, bk, wvT, out):
    from contextlib import ExitStack

    nc = tc.nc
    with ExitStack() as ctx:
        consts = ctx.enter_context(tc.tile_pool(name="consts", bufs=1))
        big = ctx.enter_context(tc.tile_pool(name="big", bufs=1))
        work = ctx.enter_context(tc.tile_pool(name="work", bufs=6))
        outp = ctx.enter_context(tc.tile_pool(name="outp", bufs=4))

        # ---- constants and big persistent SBUF tensors -----------------
        wk_sb = consts.tile([P, 2, CK], BF16)
        wv_sb = consts.tile([P, 2, CV], BF16)
        bk_sb = consts.tile([CK, 1], F32)
        mones = consts.tile([CK, 1], BF16)

        xb_sb = big.tile([P, 2, HW], BF16)
        kl = big.tile([CKE, HW], BF16)        # lhsT: [k; 1; 1]
        kq = big.tile([CKE, QH], BF16)        # rhs:  [k; -n_hi; -n_lo]
        vt8 = big.tile([P, NJ, CV + 2], FP8)  # fp8 vT; cols 256,257 = ones
        dv = big.tile([P, NQ * NIB, CV + 2], BF16)  # vT - fp8(vT), queries

        xbr = xb.rearrange("(o p) f -> p o f", p=P)

        # ---- DMA in. scalar (Act hwdge) queue: wk + first xb chunks;
        # sync queue: bk, wv + last xb chunks, then stays free for the
        # small -n row DMAs mid-prologue. Issues precede all compute on
        # their engines so the queues warm up asap.
        NXB = 8
        bs = HW // NXB
        nc.scalar.dma_start(wk_sb, wkT.rearrange("(o p) c -> p o c", p=P))
        nc.scalar.dma_start(xb_sb[:, :, 0:bs], xbr[:, :, 0:bs])
        nc.sync.dma_start(bk_sb, bk)
        nc.sync.dma_start(wv_sb, wvT.rearrange("(o p) c -> p o c", p=P))
        for t in range(1, NXB):
            eng = nc.scalar if t < 4 else nc.sync
            eng.dma_start(xb_sb[:, :, t * bs:(t + 1) * bs],
                          xbr[:, :, t * bs:(t + 1) * bs])

        # ---- memsets (after the DMA issues on their engines) -----------
        nc.vector.memset(mones, -1.0)
        nc.vector.memset(vt8[:, :, CV:CV + 2], 1.0)
        nc.gpsimd.memset(dv[:, :, CV:CV + 2], 0.0)
        nc.gpsimd.memset(kl[CK:CKE, :], 1.0)

        pool_e = ctx.enter_context(tc.tile_pool(name="pool_e", bufs=3, space="PSUM"))
        ptp = ctx.enter_context(tc.tile_pool(name="ptp", bufs=20))

        pt_pairs = {}
        ops = {}
        pool_o = [None]

        def emit_e_pair(pp):
            # pair pp = E tiles (ib, 2*jp) and (ib, 2*jp+1): same 512 query
            # cols, adjacent key chunks -> fp8 P pair laid out for PV's
            # DoubleRow lhsT.
            ib, jp = divmod(pp, NJP)
            eps = pool_e.tile([P, 2, IBS], F32, tag="e", name=f"eps_{pp}")
            for i in range(2):
                jc = 2 * jp + i
                nc.tensor.matmul(eps[:, i, :],
                                 lhsT=kl[:, jc * P:(jc + 1) * P],
                                 rhs=kq[:, ib * IBS:(ib + 1) * IBS],
                                 start=True, stop=True)
            pt = ptp.tile([P, 2, IBS], FP8, tag="pt", name=f"pt_{pp}")
            if exp_engine_for(pp) == 'a':
                nc.scalar.activation(pt, eps, EXP, bias=0.0)
            else:
                nc.vector.tensor_scalar(pt[:, :, :].bitcast(U8), eps,
                                        SCH_A, SCH_B, MUL, ADD)
            pt_pairs[pp] = pt

        def emit_pv(tick):
            # two-pass PV: pass A (q0,q1) accumulates over all 16 pairs of
            # an i-block, then pass B (q2,q3) re-reads the same pt tiles.
            # Halves concurrent PV psum tiles (2 banks) so pool_e gets 3.
            ib, tk = divmod(tick, 2 * NJP)
            pas, jp = divmod(tk, NJP)
            pp = ib * NJP + jp
            if jp == 0:
                ops[(ib, pas)] = [pool_o[0].tile([P, CV + 2], F32, tag=f"o{q}",
                                                 name=f"ops_{ib}_{pas}_{q}")
                                  for q in range(2)]
            pt = pt_pairs.pop(pp) if pas == 1 else pt_pairs[pp]
            for qi in range(2):
                q = 2 * pas + qi
                nc.tensor.matmul(ops[(ib, pas)][qi],
                                 lhsT=pt[:, :, q * P:(q + 1) * P],
                                 rhs=vt8[:, 2 * jp:2 * jp + 2, :],
                                 start=(jp == 0), stop=(jp == NJP - 1),
                                 perf_mode=DR)
                if jp == NJP - 1:
                    ic = ib * NQ + q
                    ob = outp.tile([P, CV + 2], F32, tag="ob", name=f"ob_{ic}")
                    # add the exact-diagonal fp8 repair while draining psum
                    nc.vector.tensor_tensor(ob, ops[(ib, pas)][qi],
                                            dv[:, ic, :], ADD)
                    if q % 2 == 0:
                        nc.sync.dma_start(out[ic * P:(ic + 1) * P, :], ob)
                    else:
                        nc.gpsimd.dma_start(out[ic * P:(ic + 1) * P, :], ob)

        def pair_of_tick(tick):
            ib, tk = divmod(tick, 2 * NJP)
            return ib * NJP + (tk % NJP)

        def chunk_k(t, pool_pre, pool_n):
            """k-projection chunk t -> kl (+ kq and the -n rows, t < QKC)."""
            sl = slice(t * KC, (t + 1) * KC)
            kps = pool_pre.tile([CK, KC], F32, tag="kps", name=f"kps_{t}")
            for o in range(2):
                nc.tensor.matmul(kps, lhsT=wk_sb[:, o, :],
                                 rhs=xb_sb[:, o, sl],
                                 start=(o == 0), stop=(o == 1))
            # bias adds on Act (Identity shares the Exp table set)
            nc.scalar.activation(kl[0:CK, sl], kps, IDENT, bias=bk_sb)
            if t < QKC:
                nc.scalar.activation(kq[0:CK, sl], kps, IDENT, bias=bk_sb)
                ksq = work.tile([CK, KC], BF16, tag="ksq", name=f"ksq_{t}")
                nc.vector.tensor_tensor(ksq, kq[0:CK, sl], kq[0:CK, sl], MUL)
                nps = pool_n.tile([1, KC], F32, tag="n", name=f"n_{t}")
                nc.tensor.matmul(nps, lhsT=mones, rhs=ksq,
                                 start=True, stop=True)
                # -n split into bf16 hi+lo on partition 0, then DMA'd to
                # kq partitions 64-65 (compute engines need 32-aligned
                # partition starts; DMA does not).
                nrows = work.tile([1, 2, KC], BF16, tag="nr", name=f"nr_{t}")
                nc.vector.tensor_copy(nrows[:, 0, :], nps)
                nc.vector.tensor_tensor(nrows[:, 1, :], nps,
                                        nrows[:, 0, :], SUB)
                nc.sync.dma_start(kq[CK:CKE, sl], nrows)

        # ---- prologue A: query k-proj chunks + -n rows; E starts -------
        e_next = 0
        with tc.tile_pool(name="pool_pre", bufs=1, space="PSUM") as pool_pre:
            with tc.tile_pool(name="pool_n", bufs=1, space="PSUM") as pool_n:
                for t in range(QKC):
                    chunk_k(t, pool_pre, pool_n)
                    lim = min(PRO_P, 2 * t)
                    while e_next < lim:
                        emit_e_pair(e_next)
                        e_next += 1

            # ---- prologue B: remaining k-proj + all v-proj --------------
            # two v-proj psums pack into one bank ([128, 2, 256] f32 = 2KB)
            with tc.tile_pool(name="pool_v", bufs=1, space="PSUM") as pool_v:
                for t in range(HW // KC):
                    if t >= QKC:
                        chunk_k(t, pool_pre, None)
                    for jc2 in range(t * 2, t * 2 + 2):
                        vps = pool_v.tile([P, 2, CV], F32, tag="v",
                                          name=f"vps_{jc2}")
                        for j in range(2):
                            jc = 2 * jc2 + j
                            for o in range(2):
                                nc.tensor.matmul(vps[:, j, :],
                                                 lhsT=xb_sb[:, o, jc * P:(jc + 1) * P],
                                                 rhs=wv_sb[:, o, :],
                                                 start=(o == 0), stop=(o == 1))
                            # spread psum->fp8 copies across DVE and Act
                            if jc % 2 == 0:
                                nc.vector.tensor_copy(vt8[:, jc, 0:CV],
                                                      vps[:, j, :])
                            else:
                                nc.scalar.copy(vt8[:, jc, 0:CV], vps[:, j, :])
                            if jc < NQ * NIB:
                                nc.vector.tensor_tensor(dv[:, jc, 0:CV],
                                                        vps[:, j, :],
                                                        vt8[:, jc, 0:CV], SUB)
                    lim = min(PRO_P, 2 * (t + 1))
                    while e_next < lim:
                        emit_e_pair(e_next)
                        e_next += 1

        pool_o[0] = ctx.enter_context(
            tc.tile_pool(name="pool_o", bufs=1, space="PSUM"))

        # ---- main loop: E pairs in back-to-back twos (the consecutive
        # exps land on different engines), then a paced PV-tick run ------
        total_p = NIB * NJP
        total_ticks = 2 * total_p
        n_rest = total_p - PRO_P
        pv_next = 0
        for p0 in range(PRO_P, total_p, 2):
            emit_e_pair(p0)
            emit_e_pair(p0 + 1)
            target = (p0 + 2 - PRO_P) * total_ticks // n_rest
            while pv_next < min(target, total_ticks) and \
                    pair_of_tick(pv_next) <= p0 + 1 - LAG_P:
                emit_pv(pv_next)
                pv_next += 1
        while pv_next < total_ticks:
            emit_pv(pv_next)
            pv_next += 1


def build_nc():
    nc = bacc.Bacc(trn_type="TRN2")
    xb_d = nc.dram_tensor("xb", [C, HW], BF16, kind="ExternalInput")
    wk_d = nc.dram_tensor("wkT", [C, CK], BF16, kind="ExternalInput")
    bk_d = nc.dram_tensor("bk", [CK, 1], F32, kind="ExternalInput")
    wv_d = nc.dram_tensor("wvT", [C, CV], BF16, kind="ExternalInput")
    out_d = nc.dram_tensor("out", [QH, CV + 2], F32, kind="ExternalOutput")
    args = (xb_d[:], wk_d[:], bk_d[:], wv_d[:], out_d[:])
    with tile.TileContext(nc) as tc:
        _emit(tc, *args)
    nc.finalize()
    return nc


_NC = None


def get_nc():
    global _NC
    if _NC is None:
        _NC = build_nc()
    return _NC


def build_in_maps(inputs):
    x = np.ascontiguousarray(np.asarray(inputs["x"], np.float32))
    Wk = np.asarray(inputs["Wk"], np.float32)
    bk = np.asarray(inputs["bk"], np.float32)
    gamma = np.asarray(inputs["bn_gamma"], np.float32)
    beta = np.asarray(inputs["bn_beta"], np.float32)
    mean = np.asarray(inputs["bn_mean"], np.float32)
    var = np.asarray(inputs["bn_var"], np.float32)
    Wv = np.asarray(inputs["Wv"], np.float32)

    inv = gamma / np.sqrt(var + BN_EPS)
    wk_eff = (inv[:, None] * Wk).astype(np.float32)
    bk_eff = (inv * bk + (beta - mean * inv)).astype(np.float32)

    import ml_dtypes
    bf = ml_dtypes.bfloat16
    wkT = np.ascontiguousarray(wk_eff.T.astype(bf))     # [C, CK]
    wvT = np.ascontiguousarray(Wv.T.astype(bf))         # [C, CV]
    bk2 = np.ascontiguousarray(bk_eff.reshape(CK, 1))

    in_maps = []
    for core in range(NCORES):
        b, h = divmod(core, 2)
        xf = x[b].reshape(C, HW)
        if h == 1:
            # rotate so this core's queries sit in key columns [0, QH)
            xf = np.concatenate([xf[:, QH:], xf[:, :QH]], axis=1)
        xbc = np.ascontiguousarray(xf.astype(bf))
        in_maps.append({"xb": xbc, "wkT": wkT, "bk": bk2, "wvT": wvT})
    return in_maps


def kernel(**inputs):
    bv = np.asarray(inputs["bv"], np.float32)
    in_maps = build_in_maps(inputs)
    nc = get_nc()
    res = run_bass_kernel_spmd(nc, in_maps, core_ids=list(range(NCORES)))
    out = np.empty((B, CV, HW), np.float32)
    for core in range(NCORES):
        b, h = divmod(core, 2)
        raw = res.results[core]["out"]          # [QH, CV+2]
        o = raw[:, 0:CV] / raw[:, CV:CV + 1]
        out[b, :, h * QH:(h + 1) * QH] = o.T
    out += bv[None, :, None]
    return np.ascontiguousarray(out.reshape(B, CV, HH, WW))


# revision 30
# speedup vs baseline: 1.3798x; 1.3798x over previous
"""Trainium2 Bass kernel v7 for nn_AttentionBlock (4x256x64x64 self-attention).

Sharding: 8 cores = 4 batches x 2 query-halves. Per core (batch b, half h):
  k    = fold_bn(Wk) @ x[b] + bk'       [64, 4096] keys, bf16
  n_j  = ||k_j||^2                      row max of E is always the diagonal
                                        here, so -n (split bf16 hi+lo) rides
                                        as 2 extra contraction channels
  E'   = [k;1;1]^T [k;-nhi;-nlo]        E'_ij = <k_i,k_j> - n_j  (K=66)
  P    = exp(E') in fp8 e4m3            E' <= ~0 so P in (0, ~1]; exp on Act
                                        (native, fp8 out) or DVE (Schraudolph
                                        u8 bit trick; rint+saturate verified)
  vT8  = fp8(x[b]^T @ Wv^T)             [4096, 258] fp8, cols 256/257 = ones
  num  = P^T @ vT8                      DoubleRow fp8 matmuls: 2 key chunks
                                        per instruction at 2x bf16 rate
  num += dv                             dv = vT - fp8(vT) on the query rows
                                        repairs fp8(v) on the diagonal
                                        (P_ii ~ 1); off-diag mass is ~3%
Host divides num[:, :256] by num[:, 256] (denominator), adds bv, reshapes.
Bit-level numpy sim of this pipeline: rel err 4.1e-3 (gate 2e-2).

Notes from measurement: DoubleRow streams 1 output col/cycle (2 fp8 input
cols/cycle), so it only pays when the pair dim carries two real contraction
chunks (PV); fp8 E gains nothing. --enable-ldw-opt=true crashes walrus.
GpSimd/Pool cannot touch PSUM. Compute-engine APs need 32-aligned partition
starts (the -n rows reach partitions 64-65 via a small DMA).
"""

import numpy as np

import concourse.bass as bass
import concourse.bacc as bacc
import concourse.tile as tile
import concourse.mybir as mybir
from concourse.bass_utils import run_bass_kernel_spmd

B, C, HH, WW = 4, 256, 64, 64
HW = HH * WW          # 4096
CK, CV = 64, 256
CKE = CK + 2          # 66: k channels + [-n_hi; -n_lo] (rhs) / [1; 1] (lhsT)
P = 128
QH = HW // 2          # 2048 queries per core
NCORES = 8
BN_EPS = 1e-5

NJ = HW // P          # 32 key chunks
NJP = NJ // 2         # 16 key chunk pairs (DoubleRow granule)
IBS = 512             # query columns per E tile
NIB = QH // IBS       # 4 i-blocks
NQ = IBS // P         # 4 query chunks of 128 per i-block
KC = 512              # hw chunk for the k/v projection matmuls
QKC = QH // KC        # 4 query projection chunks

LAG_P = 2             # PV lags behind exp by this many pairs
PRO_P = 12            # E pairs emitted during the projection prologue

F32 = mybir.dt.float32
BF16 = mybir.dt.bfloat16
FP8 = mybir.dt.float8e4
U8 = mybir.dt.uint8
EXP = mybir.ActivationFunctionType.Exp
IDENT = mybir.ActivationFunctionType.Identity
MUL = mybir.AluOpType.mult
ADD = mybir.AluOpType.add
SUB = mybir.AluOpType.subtract
DR = mybir.MatmulPerfMode.DoubleRow

# Schraudolph exp to e4m3 bits: u8 = rint(E*8/ln2 + (7*8 - 0.344)),
# f32->u8 cast on DVE verified on hw to round-to-nearest and saturate.
SCH_A = 8.0 / float(np.log(2.0))
SCH_B = 56.0 - 0.344


# exp engine per pair index: 'a' = Act (native Exp), 'v' = DVE (bit trick).
def exp_engine_for(pp):
    if pp < PRO_P:
        return 'v' if pp % 4 == 2 else 'a'
    return 'v' if pp % 2 == 1 else 'a'


def _emit(tc, xb, wkT, bk, wvT, out):
    from contextlib import ExitStack

    nc = tc.nc
    with ExitStack() as ctx:
        consts = ctx.enter_context(tc.tile_pool(name="consts", bufs=1))
        big = ctx.enter_context(tc.tile_pool(name="big", bufs=1))
        work = ctx.enter_context(tc.tile_pool(name="work", bufs=6))
        outp = ctx.enter_context(tc.tile_pool(name="outp", bufs=4))

        # ---- constants and big persistent SBUF tensors -----------------
        wk_sb = consts.tile([P, 2, CK], BF16)
        wv_sb = consts.tile([P, 2, CV], BF16)
        bk_sb = consts.tile([CK, 1], F32)
        mones = consts.tile([CK, 1], BF16)

        xb_sb = big.tile([P, 2, HW], BF16)
        kl = big.tile([CKE, HW], BF16)        # lhsT: [k; 1; 1]
        kq = big.tile([CKE, QH], BF16)        # rhs:  [k; -n_hi; -n_lo]
        vt8 = big.tile([P, NJ, CV + 2], FP8)  # fp8 vT; cols 256,257 = ones
        dv = big.tile([P, NQ * NIB, CV + 2], BF16)  # vT - fp8(vT), queries

        xbr = xb.rearrange("(o p) f -> p o f", p=P)

        # ---- DMA in. scalar (Act hwdge) queue: wk + first xb chunks;
        # sync queue: bk, wv + last xb chunks, then stays free for the
        # small -n row DMAs mid-prologue. Issues precede all compute on
        # their engines so the queues warm up asap.
        NXB = 8
        bs = HW // NXB
        nc.scalar.dma_start(wk_sb, wkT.rearrange("(o p) c -> p o c", p=P))
        nc.scalar.dma_start(xb_sb[:, :, 0:bs], xbr[:, :, 0:bs])
        nc.sync.dma_start(bk_sb, bk)
        nc.sync.dma_start(wv_sb, wvT.rearrange("(o p) c -> p o c", p=P))
        for t in range(1, NXB):
            eng = nc.scalar if t < 4 else nc.sync
            eng.dma_start(xb_sb[:, :, t * bs:(t + 1) * bs],
                          xbr[:, :, t * bs:(t + 1) * bs])

        # ---- memsets (after the DMA issues on their engines) -----------
        nc.vector.memset(mones, -1.0)
        nc.vector.memset(vt8[:, :, CV:CV + 2], 1.0)
        nc.gpsimd.memset(dv[:, :, CV:CV + 2], 0.0)
        nc.gpsimd.memset(kl[CK:CKE, :], 1.0)

        pool_e = ctx.enter_context(tc.tile_pool(name="pool_e", bufs=2, space="PSUM"))
        ptp = ctx.enter_context(tc.tile_pool(name="ptp", bufs=16))

        pt_pairs = {}
        ops = {}
        pool_o = [None]

        def emit_e_pair(pp):
            # pair pp = E tiles (ib, 2*jp) and (ib, 2*jp+1): same 512 query
            # cols, adjacent key chunks -> fp8 P pair laid out for PV's
            # DoubleRow lhsT.
            ib, jp = divmod(pp, NJP)
            eps = pool_e.tile([P, 2, IBS], F32, tag="e", name=f"eps_{pp}")
            for i in range(2):
                jc = 2 * jp + i
                nc.tensor.matmul(eps[:, i, :],
                                 lhsT=kl[:, jc * P:(jc + 1) * P],
                                 rhs=kq[:, ib * IBS:(ib + 1) * IBS],
                                 start=True, stop=True)
            pt = ptp.tile([P, 2, IBS], FP8, tag="pt", name=f"pt_{pp}")
            if exp_engine_for(pp) == 'a':
                nc.scalar.activation(pt, eps, EXP, bias=0.0)
            else:
                nc.vector.tensor_scalar(pt[:, :, :].bitcast(U8), eps,
                                        SCH_A, SCH_B, MUL, ADD)
            pt_pairs[pp] = pt

        def emit_pv(pp):
            ib, jp = divmod(pp, NJP)
            if jp == 0:
                ops[ib] = [pool_o[0].tile([P, CV + 2], F32, tag=f"o{q}",
                                          name=f"ops_{ib}_{q}") for q in range(NQ)]
            pt = pt_pairs.pop(pp)
            for q in range(NQ):
                nc.tensor.matmul(ops[ib][q],
                                 lhsT=pt[:, :, q * P:(q + 1) * P],
                                 rhs=vt8[:, 2 * jp:2 * jp + 2, :],
                                 start=(jp == 0), stop=(jp == NJP - 1),
                                 perf_mode=DR)
                if jp == NJP - 1:
                    ic = ib * NQ + q
                    ob = outp.tile([P, CV + 2], F32, tag="ob", name=f"ob_{ib}_{q}")
                    # add the exact-diagonal fp8 repair while draining psum
                    nc.vector.tensor_tensor(ob, ops[ib][q], dv[:, ic, :], ADD)
                    if q % 2 == 0:
                        nc.sync.dma_start(out[ic * P:(ic + 1) * P, :], ob)
                    else:
                        nc.gpsimd.dma_start(out[ic * P:(ic + 1) * P, :], ob)

        def chunk_k(t, pool_pre, pool_n):
            """k-projection chunk t -> kl (+ kq and the -n rows, t < QKC)."""
            sl = slice(t * KC, (t + 1) * KC)
            kps = pool_pre.tile([CK, KC], F32, tag="kps", name=f"kps_{t}")
            for o in range(2):
                nc.tensor.matmul(kps, lhsT=wk_sb[:, o, :],
                                 rhs=xb_sb[:, o, sl],
                                 start=(o == 0), stop=(o == 1))
            # bias adds on Act (Identity shares the Exp table set)
            nc.scalar.activation(kl[0:CK, sl], kps, IDENT, bias=bk_sb)
            if t < QKC:
                nc.scalar.activation(kq[0:CK, sl], kps, IDENT, bias=bk_sb)
                ksq = work.tile([CK, KC], BF16, tag="ksq", name=f"ksq_{t}")
                nc.vector.tensor_tensor(ksq, kq[0:CK, sl], kq[0:CK, sl], MUL)
                nps = pool_n.tile([1, KC], F32, tag="n", name=f"n_{t}")
                nc.tensor.matmul(nps, lhsT=mones, rhs=ksq,
                                 start=True, stop=True)
                # -n split into bf16 hi+lo on partition 0, then DMA'd to
                # kq partitions 64-65 (compute engines need 32-aligned
                # partition starts; DMA does not).
                nrows = work.tile([1, 2, KC], BF16, tag="nr", name=f"nr_{t}")
                nc.vector.tensor_copy(nrows[:, 0, :], nps)
                nc.vector.tensor_tensor(nrows[:, 1, :], nps,
                                        nrows[:, 0, :], SUB)
                nc.sync.dma_start(kq[CK:CKE, sl], nrows)

        # ---- prologue A: query k-proj chunks + -n rows; E starts -------
        e_next = 0
        with tc.tile_pool(name="pool_pre", bufs=2, space="PSUM") as pool_pre:
            with tc.tile_pool(name="pool_n", bufs=1, space="PSUM") as pool_n:
                for t in range(QKC):
                    chunk_k(t, pool_pre, pool_n)
                    lim = min(PRO_P, 2 * t)
                    while e_next < lim:
                        emit_e_pair(e_next)
                        e_next += 1

            # ---- prologue B: remaining k-proj + all v-proj --------------
            with tc.tile_pool(name="pool_v", bufs=2, space="PSUM") as pool_v:
                for t in range(HW // KC):
                    if t >= QKC:
                        chunk_k(t, pool_pre, None)
                    for jc in range(t * 4, t * 4 + 4):
                        vps = pool_v.tile([P, CV], F32, tag="v", name=f"vps_{jc}")
                        for o in range(2):
                            nc.tensor.matmul(vps,
                                             lhsT=xb_sb[:, o, jc * P:(jc + 1) * P],
                                             rhs=wv_sb[:, o, :],
                                             start=(o == 0), stop=(o == 1))
                        # spread psum->fp8 copies across DVE and Act
                        if jc % 2 == 0:
                            nc.vector.tensor_copy(vt8[:, jc, 0:CV], vps)
                        else:
                            nc.scalar.copy(vt8[:, jc, 0:CV], vps)
                        if jc < NQ * NIB:
                            nc.vector.tensor_tensor(dv[:, jc, 0:CV], vps,
                                                    vt8[:, jc, 0:CV], SUB)
                    lim = min(PRO_P, 2 * (t + 1))
                    while e_next < lim:
                        emit_e_pair(e_next)
                        e_next += 1

        pool_o[0] = ctx.enter_context(
            tc.tile_pool(name="pool_o", bufs=1, space="PSUM"))

        # ---- main loop: E pairs in back-to-back twos (the consecutive
        # exps land on different engines), then a paced PV run -----------
        total_p = NIB * NJP
        n_rest = total_p - PRO_P
        pv_next = 0
        for p0 in range(PRO_P, total_p, 2):
            emit_e_pair(p0)
            emit_e_pair(p0 + 1)
            target = (p0 + 2 - PRO_P) * total_p // n_rest
            while pv_next < min(target, p0 + 2 - LAG_P):
                emit_pv(pv_next)
                pv_next += 1
        while pv_next < total_p:
            emit_pv(pv_next)
            pv_next += 1


def build_nc():
    nc = bacc.Bacc(trn_type="TRN2")
    xb_d = nc.dram_tensor("xb", [C, HW], BF16, kind="ExternalInput")
    wk_d = nc.dram_tensor("wkT", [C, CK], BF16, kind="ExternalInput")
    bk_d = nc.dram_tensor("bk", [CK, 1], F32, kind="ExternalInput")
    wv_d = nc.dram_tensor("wvT", [C, CV], BF16, kind="ExternalInput")
    out_d = nc.dram_tensor("out", [QH, CV + 2], F32, kind="ExternalOutput")
    args = (xb_d[:], wk_d[:], bk_d[:], wv_d[:], out_d[:])
    with tile.TileContext(nc) as tc:
        _emit(tc, *args)
    nc.finalize()
    return nc


_NC = None


def get_nc():
    global _NC
    if _NC is None:
        _NC = build_nc()
    return _NC


def build_in_maps(inputs):
    x = np.ascontiguousarray(np.asarray(inputs["x"], np.float32))
    Wk = np.asarray(inputs["Wk"], np.float32)
    bk = np.asarray(inputs["bk"], np.float32)
    gamma = np.asarray(inputs["bn_gamma"], np.float32)
    beta = np.asarray(inputs["bn_beta"], np.float32)
    mean = np.asarray(inputs["bn_mean"], np.float32)
    var = np.asarray(inputs["bn_var"], np.float32)
    Wv = np.asarray(inputs["Wv"], np.float32)

    inv = gamma / np.sqrt(var + BN_EPS)
    wk_eff = (inv[:, None] * Wk).astype(np.float32)
    bk_eff = (inv * bk + (beta - mean * inv)).astype(np.float32)

    import ml_dtypes
    bf = ml_dtypes.bfloat16
    wkT = np.ascontiguousarray(wk_eff.T.astype(bf))     # [C, CK]
    wvT = np.ascontiguousarray(Wv.T.astype(bf))         # [C, CV]
    bk2 = np.ascontiguousarray(bk_eff.reshape(CK, 1))

    in_maps = []
    for core in range(NCORES):
        b, h = divmod(core, 2)
        xf = x[b].reshape(C, HW)
        if h == 1:
            # rotate so this core's queries sit in key columns [0, QH)
            xf = np.concatenate([xf[:, QH:], xf[:, :QH]], axis=1)
        xbc = np.ascontiguousarray(xf.astype(bf))
        in_maps.append({"xb": xbc, "wkT": wkT, "bk": bk2, "wvT": wvT})
    return in_maps


def kernel(**inputs):
    bv = np.asarray(inputs["bv"], np.float32)
    in_maps = build_in_maps(inputs)
    nc = get_nc()
    res = run_bass_kernel_spmd(nc, in_maps, core_ids=list(range(NCORES)))
    out = np.empty((B, CV, HW), np.float32)
    for core in range(NCORES):
        b, h = divmod(core, 2)
        raw = res.results[core]["out"]          # [QH, CV+2]
        o = raw[:, 0:CV] / raw[:, CV:CV + 1]
        out[b, :, h * QH:(h + 1) * QH] = o.T
    out += bv[None, :, None]
    return np.ascontiguousarray(out.reshape(B, CV, HH, WW))


# revision 31
# speedup vs baseline: 1.4243x; 1.0323x over previous
"""Trainium2 Bass kernel v7 for nn_AttentionBlock (4x256x64x64 self-attention).

Sharding: 8 cores = 4 batches x 2 query-halves. Per core (batch b, half h):
  k    = fold_bn(Wk) @ x[b] + bk'       [64, 4096] keys, bf16
  n_j  = ||k_j||^2                      row max of E is always the diagonal
                                        here, so -n (split bf16 hi+lo) rides
                                        as 2 extra contraction channels
  E'   = [k;1;1]^T [k;-nhi;-nlo]        E'_ij = <k_i,k_j> - n_j  (K=66)
  P    = exp(E') in fp8 e4m3            E' <= ~0 so P in (0, ~1]; exp on Act
                                        (native, fp8 out) or DVE (Schraudolph
                                        u8 bit trick; rint+saturate verified)
  vT8  = fp8(x[b]^T @ Wv^T)             [4096, 258] fp8, cols 256/257 = ones
  num  = P^T @ vT8                      DoubleRow fp8 matmuls: 2 key chunks
                                        per instruction at 2x bf16 rate
  num += dv                             dv = vT - fp8(vT) on the query rows
                                        repairs fp8(v) on the diagonal
                                        (P_ii ~ 1); off-diag mass is ~3%
Host divides num[:, :256] by num[:, 256] (denominator), adds bv, reshapes.
Bit-level numpy sim of this pipeline: rel err 4.1e-3 (gate 2e-2).

Notes from measurement: DoubleRow streams 1 output col/cycle (2 fp8 input
cols/cycle), so it only pays when the pair dim carries two real contraction
chunks (PV); fp8 E gains nothing. --enable-ldw-opt=true crashes walrus.
GpSimd/Pool cannot touch PSUM. Compute-engine APs need 32-aligned partition
starts (the -n rows reach partitions 64-65 via a small DMA).
"""

import numpy as np

import concourse.bass as bass
import concourse.bacc as bacc
import concourse.tile as tile
import concourse.mybir as mybir
from concourse.bass_utils import run_bass_kernel_spmd

B, C, HH, WW = 4, 256, 64, 64
HW = HH * WW          # 4096
CK, CV = 64, 256
CKE = CK + 2          # 66: k channels + [-n_hi; -n_lo] (rhs) / [1; 1] (lhsT)
P = 128
QH = HW // 2          # 2048 queries per core
NCORES = 8
BN_EPS = 1e-5

NJ = HW // P          # 32 key chunks
NJP = NJ // 2         # 16 key chunk pairs (DoubleRow granule)
IBS = 512             # query columns per E tile
NIB = QH // IBS       # 4 i-blocks
NQ = IBS // P         # 4 query chunks of 128 per i-block
KC = 512              # hw chunk for the k/v projection matmuls
QKC = QH // KC        # 4 query projection chunks

LAG_P = 2             # PV lags behind exp by this many pairs
PRO_P = 12            # E pairs emitted during the projection prologue

F32 = mybir.dt.float32
BF16 = mybir.dt.bfloat16
FP8 = mybir.dt.float8e4
U8 = mybir.dt.uint8
EXP = mybir.ActivationFunctionType.Exp
IDENT = mybir.ActivationFunctionType.Identity
MUL = mybir.AluOpType.mult
ADD = mybir.AluOpType.add
SUB = mybir.AluOpType.subtract
DR = mybir.MatmulPerfMode.DoubleRow

# Schraudolph exp to e4m3 bits: u8 = rint(E*8/ln2 + (7*8 - 0.344)),
# f32->u8 cast on DVE verified on hw to round-to-nearest and saturate.
SCH_A = 8.0 / float(np.log(2.0))
SCH_B = 56.0 - 0.344


# exp engine per pair index: 'a' = Act (native Exp), 'v' = DVE (bit trick).
def exp_engine_for(pp):
    if pp < PRO_P:
        return 'v' if pp % 4 == 2 else 'a'
    return 'v' if pp % 2 == 1 else 'a'


def _emit(tc, xb, wkT, bk, wvT, out):
    from contextlib import ExitStack

    nc = tc.nc
    with ExitStack() as ctx:
        consts = ctx.enter_context(tc.tile_pool(name="consts", bufs=1))
        big = ctx.enter_context(tc.tile_pool(name="big", bufs=1))
        work = ctx.enter_context(tc.tile_pool(name="work", bufs=6))
        outp = ctx.enter_context(tc.tile_pool(name="outp", bufs=4))

        # ---- constants and big persistent SBUF tensors -----------------
        wk_sb = consts.tile([P, 2, CK], BF16)
        wv_sb = consts.tile([P, 2, CV], BF16)
        bk_sb = consts.tile([CK, 1], F32)
        mones = consts.tile([CK, 1], BF16)

        xb_sb = big.tile([P, 2, HW], BF16)
        kl = big.tile([CKE, HW], BF16)        # lhsT: [k; 1; 1]
        kq = big.tile([CKE, QH], BF16)        # rhs:  [k; -n_hi; -n_lo]
        vt8 = big.tile([P, NJ, CV + 2], FP8)  # fp8 vT; cols 256,257 = ones
        dv = big.tile([P, NQ * NIB, CV + 2], BF16)  # vT - fp8(vT), queries

        xbr = xb.rearrange("(o p) f -> p o f", p=P)

        # ---- DMA in. scalar (Act hwdge) queue: wk + first xb chunks;
        # sync queue: bk, wv + last xb chunks, then stays free for the
        # small -n row DMAs mid-prologue. Issues precede all compute on
        # their engines so the queues warm up asap.
        NXB = 8
        bs = HW // NXB
        nc.scalar.dma_start(wk_sb, wkT.rearrange("(o p) c -> p o c", p=P))
        nc.scalar.dma_start(xb_sb[:, :, 0:bs], xbr[:, :, 0:bs])
        nc.sync.dma_start(bk_sb, bk)
        nc.sync.dma_start(wv_sb, wvT.rearrange("(o p) c -> p o c", p=P))
        for t in range(1, NXB):
            eng = nc.scalar if t < 4 else nc.sync
            eng.dma_start(xb_sb[:, :, t * bs:(t + 1) * bs],
                          xbr[:, :, t * bs:(t + 1) * bs])

        # ---- memsets (after the DMA issues on their engines) -----------
        nc.vector.memset(mones, -1.0)
        nc.vector.memset(vt8[:, :, CV:CV + 2], 1.0)
        nc.gpsimd.memset(dv[:, :, CV:CV + 2], 0.0)
        nc.gpsimd.memset(kl[CK:CKE, :], 1.0)

        pool_e = ctx.enter_context(tc.tile_pool(name="pool_e", bufs=2, space="PSUM"))
        ptp = ctx.enter_context(tc.tile_pool(name="ptp", bufs=16))

        pt_pairs = {}
        ops = {}
        pool_o = [None]

        def emit_e_pair(pp):
            # pair pp = E tiles (ib, 2*jp) and (ib, 2*jp+1): same 512 query
            # cols, adjacent key chunks -> fp8 P pair laid out for PV's
            # DoubleRow lhsT.
            ib, jp = divmod(pp, NJP)
            eps = pool_e.tile([P, 2, IBS], F32, tag="e", name=f"eps_{pp}")
            for i in range(2):
                jc = 2 * jp + i
                nc.tensor.matmul(eps[:, i, :],
                                 lhsT=kl[:, jc * P:(jc + 1) * P],
                                 rhs=kq[:, ib * IBS:(ib + 1) * IBS],
                                 start=True, stop=True)
            pt = ptp.tile([P, 2, IBS], FP8, tag="pt", name=f"pt_{pp}")
            if exp_engine_for(pp) == 'a':
                nc.scalar.activation(pt, eps, EXP, bias=0.0)
            else:
                nc.vector.tensor_scalar(pt[:, :, :].bitcast(U8), eps,
                                        SCH_A, SCH_B, MUL, ADD)
            pt_pairs[pp] = pt

        def emit_pv(pp):
            ib, jp = divmod(pp, NJP)
            if jp == 0:
                ops[ib] = [pool_o[0].tile([P, CV + 2], F32, tag=f"o{q}",
                                          name=f"ops_{ib}_{q}") for q in range(NQ)]
            pt = pt_pairs.pop(pp)
            for q in range(NQ):
                nc.tensor.matmul(ops[ib][q],
                                 lhsT=pt[:, :, q * P:(q + 1) * P],
                                 rhs=vt8[:, 2 * jp:2 * jp + 2, :],
                                 start=(jp == 0), stop=(jp == NJP - 1),
                                 perf_mode=DR)
                if jp == NJP - 1:
                    ic = ib * NQ + q
                    ob = outp.tile([P, CV + 2], F32, tag="ob", name=f"ob_{ib}_{q}")
                    # add the exact-diagonal fp8 repair while draining psum
                    nc.vector.tensor_tensor(ob, ops[ib][q], dv[:, ic, :], ADD)
                    if q % 2 == 0:
                        nc.sync.dma_start(out[ic * P:(ic + 1) * P, :], ob)
                    else:
                        nc.gpsimd.dma_start(out[ic * P:(ic + 1) * P, :], ob)

        def chunk_k(t, pool_pre, pool_n):
            """k-projection chunk t -> kl (+ kq and the -n rows, t < QKC)."""
            sl = slice(t * KC, (t + 1) * KC)
            kps = pool_pre.tile([CK, KC], F32, tag="kps", name=f"kps_{t}")
            for o in range(2):
                nc.tensor.matmul(kps, lhsT=wk_sb[:, o, :],
                                 rhs=xb_sb[:, o, sl],
                                 start=(o == 0), stop=(o == 1))
            # bias adds on Act (Identity shares the Exp table set)
            nc.scalar.activation(kl[0:CK, sl], kps, IDENT, bias=bk_sb)
            if t < QKC:
                nc.scalar.activation(kq[0:CK, sl], kps, IDENT, bias=bk_sb)
                ksq = work.tile([CK, KC], BF16, tag="ksq", name=f"ksq_{t}")
                nc.vector.tensor_tensor(ksq, kq[0:CK, sl], kq[0:CK, sl], MUL)
                nps = pool_n.tile([1, KC], F32, tag="n", name=f"n_{t}")
                nc.tensor.matmul(nps, lhsT=mones, rhs=ksq,
                                 start=True, stop=True)
                # -n split into bf16 hi+lo on partition 0, then DMA'd to
                # kq partitions 64-65 (compute engines need 32-aligned
                # partition starts; DMA does not).
                nrows = work.tile([1, 2, KC], BF16, tag="nr", name=f"nr_{t}")
                nc.vector.tensor_copy(nrows[:, 0, :], nps)
                nc.vector.tensor_tensor(nrows[:, 1, :], nps,
                                        nrows[:, 0, :], SUB)
                nc.sync.dma_start(kq[CK:CKE, sl], nrows)

        # ---- PE warm-up: the tensor engine ramps 0.65 -> 2.4 GHz over
        # ~3us of continuous work. It would otherwise ramp through the
        # latency-critical k-proj/E prologue; burn the input-DMA wait
        # (~9us) on dummy matmuls over zeroed tiles instead.
        wa = work.tile([P, 1], BF16, tag="wa", name="warm_a")
        wb = work.tile([P, KC], BF16, tag="wb", name="warm_b")
        nc.vector.memset(wa, 0.0)
        nc.vector.memset(wb, 0.0)
        with tc.tile_pool(name="pool_w", bufs=1, space="PSUM") as pool_w:
            for i in range(14):
                wps = pool_w.tile([1, KC], F32, tag="w", name=f"warm_{i}")
                nc.tensor.matmul(wps, lhsT=wa, rhs=wb, start=True, stop=True)

        # ---- prologue A: query k-proj chunks + -n rows; E starts -------
        e_next = 0
        with tc.tile_pool(name="pool_pre", bufs=2, space="PSUM") as pool_pre:
            with tc.tile_pool(name="pool_n", bufs=1, space="PSUM") as pool_n:
                for t in range(QKC):
                    chunk_k(t, pool_pre, pool_n)
                    lim = min(PRO_P, 2 * t)
                    while e_next < lim:
                        emit_e_pair(e_next)
                        e_next += 1

            # ---- prologue B: remaining k-proj + all v-proj --------------
            with tc.tile_pool(name="pool_v", bufs=2, space="PSUM") as pool_v:
                for t in range(HW // KC):
                    if t >= QKC:
                        chunk_k(t, pool_pre, None)
                    for jc in range(t * 4, t * 4 + 4):
                        vps = pool_v.tile([P, CV], F32, tag="v", name=f"vps_{jc}")
                        for o in range(2):
                            nc.tensor.matmul(vps,
                                             lhsT=xb_sb[:, o, jc * P:(jc + 1) * P],
                                             rhs=wv_sb[:, o, :],
                                             start=(o == 0), stop=(o == 1))
                        # spread psum->fp8 copies across DVE and Act
                        if jc % 2 == 0:
                            nc.vector.tensor_copy(vt8[:, jc, 0:CV], vps)
                        else:
                            nc.scalar.copy(vt8[:, jc, 0:CV], vps)
                        if jc < NQ * NIB:
                            nc.vector.tensor_tensor(dv[:, jc, 0:CV], vps,
                                                    vt8[:, jc, 0:CV], SUB)
                    lim = min(PRO_P, 2 * (t + 1))
                    while e_next < lim:
                        emit_e_pair(e_next)
                        e_next += 1

        pool_o[0] = ctx.enter_context(
            tc.tile_pool(name="pool_o", bufs=1, space="PSUM"))

        # ---- main loop: E pairs in back-to-back twos (the consecutive
        # exps land on different engines), then a paced PV run -----------
        total_p = NIB * NJP
        n_rest = total_p - PRO_P
        pv_next = 0
        for p0 in range(PRO_P, total_p, 2):
            emit_e_pair(p0)
            emit_e_pair(p0 + 1)
            target = (p0 + 2 - PRO_P) * total_p // n_rest
            while pv_next < min(target, p0 + 2 - LAG_P):
                emit_pv(pv_next)
                pv_next += 1
        while pv_next < total_p:
            emit_pv(pv_next)
            pv_next += 1


def build_nc():
    nc = bacc.Bacc(trn_type="TRN2")
    xb_d = nc.dram_tensor("xb", [C, HW], BF16, kind="ExternalInput")
    wk_d = nc.dram_tensor("wkT", [C, CK], BF16, kind="ExternalInput")
    bk_d = nc.dram_tensor("bk", [CK, 1], F32, kind="ExternalInput")
    wv_d = nc.dram_tensor("wvT", [C, CV], BF16, kind="ExternalInput")
    out_d = nc.dram_tensor("out", [QH, CV + 2], F32, kind="ExternalOutput")
    args = (xb_d[:], wk_d[:], bk_d[:], wv_d[:], out_d[:])
    with tile.TileContext(nc) as tc:
        _emit(tc, *args)
    nc.finalize()
    return nc


_NC = None


def get_nc():
    global _NC
    if _NC is None:
        _NC = build_nc()
    return _NC


def build_in_maps(inputs):
    x = np.ascontiguousarray(np.asarray(inputs["x"], np.float32))
    Wk = np.asarray(inputs["Wk"], np.float32)
    bk = np.asarray(inputs["bk"], np.float32)
    gamma = np.asarray(inputs["bn_gamma"], np.float32)
    beta = np.asarray(inputs["bn_beta"], np.float32)
    mean = np.asarray(inputs["bn_mean"], np.float32)
    var = np.asarray(inputs["bn_var"], np.float32)
    Wv = np.asarray(inputs["Wv"], np.float32)

    inv = gamma / np.sqrt(var + BN_EPS)
    wk_eff = (inv[:, None] * Wk).astype(np.float32)
    bk_eff = (inv * bk + (beta - mean * inv)).astype(np.float32)

    import ml_dtypes
    bf = ml_dtypes.bfloat16
    wkT = np.ascontiguousarray(wk_eff.T.astype(bf))     # [C, CK]
    wvT = np.ascontiguousarray(Wv.T.astype(bf))         # [C, CV]
    bk2 = np.ascontiguousarray(bk_eff.reshape(CK, 1))

    in_maps = []
    for core in range(NCORES):
        b, h = divmod(core, 2)
        xf = x[b].reshape(C, HW)
        if h == 1:
            # rotate so this core's queries sit in key columns [0, QH)
            xf = np.concatenate([xf[:, QH:], xf[:, :QH]], axis=1)
        xbc = np.ascontiguousarray(xf.astype(bf))
        in_maps.append({"xb": xbc, "wkT": wkT, "bk": bk2, "wvT": wvT})
    return in_maps


def kernel(**inputs):
    bv = np.asarray(inputs["bv"], np.float32)
    in_maps = build_in_maps(inputs)
    nc = get_nc()
    res = run_bass_kernel_spmd(nc, in_maps, core_ids=list(range(NCORES)))
    out = np.empty((B, CV, HW), np.float32)
    for core in range(NCORES):
        b, h = divmod(core, 2)
        raw = res.results[core]["out"]          # [QH, CV+2]
        o = raw[:, 0:CV] / raw[:, CV:CV + 1]
        out[b, :, h * QH:(h + 1) * QH] = o.T
    out += bv[None, :, None]
    return np.ascontiguousarray(out.reshape(B, CV, HH, WW))
